# revision 1
# baseline (speedup 1.0000x reference)
"""Bloom self-attention (fused QKV + causal softmax attention) on 8 TRN2 cores.

Sharding: core c handles batch b=c//2 and head-group hg=c%2 (8 of 16 heads).
Each core computes QKV projection for its columns of W, then causal attention
for its 8 heads, writing out[s, 1024] (fp32). Host transposes/casts/slices
inputs and gathers outputs.

Layout notes (per core, on device):
  xt   [16,128,2048] bf16  : X_b^T d-tiles (d on partitions)
  wq/wk[8,128,2048]  bf16  : per head, stationary W tiles (partition-major)
  wv   [2,128,8192]  bf16  : per head-quad, 4 heads' v columns batched
  bq/bk[128,8]        f32  : per-partition bias columns per head
  bvq  [2,128,520]    bf16 : v-bias rows replicated + interleaved 1.0 columns
                             (130-stride: per quad-head 128 v cols, a ones col,
                              a zero pad col) -> attn@[v|1] yields rowsums
  mask [128,896]      bf16 : causal staircase; slice [:, 384-r:384-r+512] is
                             the 0/1 mask for a diagonal block at offset r

The emission order software-pipelines PE-dense work (QKV/V matmul chunks)
against ACT-bound attention chunks of the previous head, so the Tile
scheduler can fill exp-wait PE gaps with projection matmuls.
"""

import math
from contextlib import ExitStack

import numpy as np
import ml_dtypes

import concourse.mybir as mybir
import concourse.tile as tile
from concourse import bacc
from concourse.bass_utils import run_bass_kernel_spmd

B, S, D = 4, 2048, 2048
H, HD = 16, 128
N_CORES = 8
DT = D // 128   # 16 d-tiles
NQB = S // 512  # 4 q-blocks
SCALE = 1.0 / math.sqrt(HD)

BF16 = mybir.dt.bfloat16
F32 = mybir.dt.float32
BF16_NP = ml_dtypes.bfloat16


def build_nc(repeat: int = 1):
    nc = bacc.Bacc(
        "TRN2",
        target_bir_lowering=False,
        debug=False,
        enable_asserts=False,
        num_devices=N_CORES,
    )
    xt_d = nc.dram_tensor("xt", [DT, 128, S], BF16, kind="ExternalInput")
    wq_d = nc.dram_tensor("wq", [8, 128, DT * 128], BF16, kind="ExternalInput")
    wk_d = nc.dram_tensor("wk", [8, 128, DT * 128], BF16, kind="ExternalInput")
    wv_d = nc.dram_tensor("wv", [2, 128, DT * 512], BF16, kind="ExternalInput")
    bq_d = nc.dram_tensor("bq", [128, 8], F32, kind="ExternalInput")
    bk_d = nc.dram_tensor("bk", [128, 8], F32, kind="ExternalInput")
    bvq_d = nc.dram_tensor("bvq", [2, 128, 520], BF16, kind="ExternalInput")
    mask_d = nc.dram_tensor("mask", [128, 896], BF16, kind="ExternalInput")
    out_d = nc.dram_tensor("out", [S, 1024], F32, kind="ExternalOutput")

    with ExitStack() as ctx:
        tc = ctx.enter_context(tile.TileContext(nc))
        singles = ctx.enter_context(tc.tile_pool(name="singles", bufs=1))
        wqk_pool = ctx.enter_context(tc.tile_pool(name="wqk", bufs=2))
        wv_pool = ctx.enter_context(tc.tile_pool(name="wvp", bufs=2))
        qk_pool = ctx.enter_context(tc.tile_pool(name="qk", bufs=2))
        v4_pool = ctx.enter_context(tc.tile_pool(name="v4", bufs=2))
        p_pool = ctx.enter_context(tc.tile_pool(name="pp", bufs=24))
        o_pool = ctx.enter_context(tc.tile_pool(name="op", bufs=8))
        r_pool = ctx.enter_context(tc.tile_pool(name="rp", bufs=8))
        ps_big = ctx.enter_context(tc.tile_pool(name="ps_big", bufs=7, space="PSUM"))
        ps_out = ctx.enter_context(tc.tile_pool(name="ps_out", bufs=1, space="PSUM"))

        # ---- resident constants (loaded once) ----
        # Interleave wv(0) chunk DMAs with the xt tile DMAs: the first
        # (dt-outer) v-chunk consumes exactly wv[:, dt*512:...]+xt[dt] per
        # step, so the PE ramp starts as soon as the first pair lands.
        wv_first = wv_pool.tile([128, DT * 512], BF16, tag="wv")
        xt = []
        for dt in range(DT):
            nc.sync.dma_start(
                out=wv_first[:, dt * 512 : (dt + 1) * 512],
                in_=wv_d.ap()[0, :, dt * 512 : (dt + 1) * 512],
            )
            t = singles.tile([128, S], BF16, tag=f"xt{dt}")
            nc.sync.dma_start(out=t[:], in_=xt_d.ap()[dt, :, :])
            xt.append(t)
        mask = singles.tile([128, 896], BF16, tag="mask")
        nc.sync.dma_start(out=mask[:], in_=mask_d.ap())
        bvq = singles.tile([128, 2 * 520], BF16, tag="bvq")
        for g in range(2):
            nc.sync.dma_start(
                out=bvq[:, g * 520 : (g + 1) * 520], in_=bvq_d.ap()[g, :, :]
            )
        bq = singles.tile([128, 8], F32, tag="bq")
        nc.sync.dma_start(out=bq[:], in_=bq_d.ap())
        bk = singles.tile([128, 8], F32, tag="bk")
        nc.sync.dma_start(out=bk[:], in_=bk_d.ap())
        # prewarm the ACT exp table set (~2.7us PSEUDO_LOAD on first Exp)
        # while the startup DMAs run, instead of inside the first attention
        # chain
        warm = singles.tile([128, 1], F32, tag="warm")
        nc.vector.memset(warm[:], 0.0)
        nc.scalar.activation(warm[:], warm[:], mybir.ActivationFunctionType.Exp)

        for _rep in range(repeat):
            # per-rep state: tiles keyed by quad / head
            v4s = {}     # g -> [16 v4 tiles]
            wv_gs = {}   # g -> wv tile
            qks = {}     # h -> (qT, kT, wq_h, wk_h)

            def v_start(g):
                if g == 0 and _rep == 0:
                    wv_g = wv_first
                else:
                    wv_g = wv_pool.tile([128, DT * 512], BF16, tag="wv")
                    nc.sync.dma_start(out=wv_g[:], in_=wv_d.ap()[g, :, :])
                wv_gs[g] = wv_g
                v4s[g] = []

            def v_chunk(g, sts):
                """v4[st] = X @ Wv_quad + bv (+ interleaved ones cols).

                dt-outer over the st group so each xt[dt] tile is consumed as
                soon as its DMA lands (matters for the startup ramp)."""
                wv_g = wv_gs[g]
                sts = list(sts)
                psvs = []
                for st in sts:
                    psv = ps_big.tile([128, 512], F32, tag="ps_big")
                    psvs.append(psv)
                for dt in range(DT):
                    for st, psv in zip(sts, psvs):
                        nc.tensor.matmul(
                            psv[:],
                            lhsT=xt[dt][:, st * 128 : (st + 1) * 128],
                            rhs=wv_g[:, dt * 512 : (dt + 1) * 512],
                            start=(dt == 0),
                            stop=(dt == DT - 1),
                        )
                for st, psv in zip(sts, psvs):
                    v4t = v4_pool.tile([128, 520], BF16, tag=f"v4_{st}")
                    nc.vector.tensor_copy(v4t[:], bvq[:, g * 520 : (g + 1) * 520])
                    dst = v4t[:].rearrange("p (q c) -> p q c", q=4)[:, :, 0:128]
                    src = psv[:].rearrange("p (q c) -> p q c", q=4)
                    nc.vector.tensor_add(dst, dst, src)
                    v4s[g].append(v4t)

            def qkv_start(h):
                wq_h = wqk_pool.tile([128, DT * 128], BF16, tag="wq")
                nc.sync.dma_start(out=wq_h[:], in_=wq_d.ap()[h, :, :])
                wk_h = wqk_pool.tile([128, DT * 128], BF16, tag="wk")
                nc.sync.dma_start(out=wk_h[:], in_=wk_d.ap()[h, :, :])
                qT = qk_pool.tile([128, S], BF16, tag="qT")
                kT = qk_pool.tile([128, S], BF16, tag="kT")
                qks[h] = (qT, kT, wq_h, wk_h)

            def qkv_chunk(h, sb):
                """qT/kT columns for s-block sb of head h."""
                qT, kT, wq_h, wk_h = qks[h]
                for w_h, dest, bias in ((wq_h, qT, bq), (wk_h, kT, bk)):
                    psx = ps_big.tile([128, 512], F32, tag="ps_big")
                    for dt in range(DT):
                        nc.tensor.matmul(
                            psx[:],
                            lhsT=w_h[:, dt * 128 : (dt + 1) * 128],
                            rhs=xt[dt][:, sb * 512 : (sb + 1) * 512],
                            start=(dt == 0),
                            stop=(dt == DT - 1),
                        )
                    nc.vector.tensor_scalar_add(
                        dest[:, sb * 512 : (sb + 1) * 512], psx[:], bias[:, h : h + 1]
                    )

            attn_ps = {}  # (h, qb) -> [(p_tile, off)]

            def attn_scores(h, qb, lo=0, hi=None):
                """Scores + exp (+causal mask) for q-block qb of head h.

                Diagonal k-tiles are trimmed to their live width: tile kt
                covers q_local in [off, 512) with off = max(kt*128-qb*512, 0).
                """
                qT, kT = qks[h][0], qks[h][1]
                n_kt = 4 * qb + 4
                if hi is None:
                    hi = n_kt
                ps = attn_ps.setdefault((h, qb), [])
                for kt in range(lo, hi):
                    r = kt * 128 - qb * 512
                    off = max(r, 0)
                    nw = 512 - off
                    pss = ps_big.tile([128, 512], F32, tag="ps_big")
                    nc.tensor.matmul(
                        pss[:, 0:nw],
                        lhsT=kT[:, kt * 128 : (kt + 1) * 128],
                        rhs=qT[:, qb * 512 + off : (qb + 1) * 512],
                        start=True,
                        stop=True,
                    )
                    p_sb = p_pool.tile([128, 512], BF16, tag="p")
                    nc.scalar.activation(
                        p_sb[:, 0:nw], pss[:, 0:nw],
                        mybir.ActivationFunctionType.Exp, scale=SCALE,
                    )
                    if r >= 0:  # diagonal block: apply causal 0/1 mask
                        nc.vector.tensor_mul(
                            p_sb[:, 0:nw], p_sb[:, 0:nw], mask[:, 384 : 384 + nw]
                        )
                    ps.append((p_sb, off))

            def attn_out(h, qb):
                """attn @ [v|1], normalize, and store, for q-block qb."""
                g, hq = h // 4, h % 4
                v4 = v4s[g]
                ps = attn_ps.pop((h, qb))
                for j in range(4):
                    poj = ps_out.tile([128, 129], F32, tag="po")
                    last_kt = 4 * qb + j  # causality: kt*128 <= qb*512 + j*128
                    for kt in range(last_kt + 1):
                        p_sb, off = ps[kt]
                        nc.tensor.matmul(
                            poj[:],
                            lhsT=p_sb[:, j * 128 - off : j * 128 - off + 128],
                            rhs=v4[kt][:, hq * 130 : hq * 130 + 129],
                            start=(kt == 0),
                            stop=(kt == last_kt),
                        )
                    recip = r_pool.tile([128, 1], F32, tag="recip")
                    nc.vector.reciprocal(recip[:], poj[:, 128:129])
                    o_sb = o_pool.tile([128, 128], F32, tag="o")
                    nc.vector.tensor_scalar_mul(o_sb[:], poj[:, 0:128], recip[:])
                    nc.sync.dma_start(
                        out=out_d.ap()[
                            qb * 512 + j * 128 : qb * 512 + (j + 1) * 128,
                            h * 128 : (h + 1) * 128,
                        ],
                        in_=o_sb[:],
                    )

            # ---- software-pipelined emission ----
            # Each step pairs a PE-dense item (4 chunks) with the attention of
            # an already-projected head: scores(qb) -> pe chunk -> out(qb), so
            # exps for qb run on ACT while PE does projection matmuls.
            def v_item(g):
                v_start(g)
                return [lambda i=i: v_chunk(g, range(4 * i, 4 * i + 4))
                        for i in range(4)]

            def qkv_item(h):
                qkv_start(h)
                return [lambda sb=sb: qkv_chunk(h, sb) for sb in range(NQB)]

            steps = [
                (lambda: v_item(0), None),
                (lambda: qkv_item(0), None),
                (lambda: qkv_item(1), 0),
                (lambda: qkv_item(2), 1),
                (lambda: qkv_item(3), 2),
                (lambda: qkv_item(4), 3),
                (lambda: v_item(1), None),
                (lambda: qkv_item(5), 4),
                (lambda: qkv_item(6), 5),
                (lambda: qkv_item(7), 6),
                (None, 7),
            ]
            for pe_item, h_attn in steps:
                pe_chunks = pe_item() if pe_item is not None else [None] * NQB
                _lo = {0: 0, 1: 2, 2: 4, 3: 5}
                for i in range(NQB):
                    if h_attn is not None:
                        attn_scores(h_attn, i, lo=_lo[i])
                    if pe_chunks[i] is not None:
                        pe_chunks[i]()
                    if h_attn is not None:
                        if i + 1 < NQB and _lo[i + 1] > 0:
                            attn_scores(h_attn, i + 1, lo=0, hi=_lo[i + 1])
                        attn_out(h_attn, i)
    nc.compile()
    return nc


def make_in_maps(hidden_states, W, b):
    """Host-side sharding: slice/transpose/cast inputs per core."""
    X = np.asarray(hidden_states, dtype=np.float32)
    Wf = np.asarray(W, dtype=np.float32).reshape(D, D, 3)
    bf = np.asarray(b, dtype=np.float32).reshape(D, 3)

    # causal staircase mask: mask[p, c] = 1 if c >= p + 384
    cols = np.arange(896)[None, :]
    rows = np.arange(128)[:, None]
    mask = (cols >= rows + 384).astype(BF16_NP)

    in_maps = []
    for c in range(N_CORES):
        bcore, hg = c // 2, c % 2
        dm0 = hg * 1024
        xt = np.ascontiguousarray(X[bcore].T).reshape(DT, 128, S).astype(BF16_NP)
        wq = np.ascontiguousarray(
            Wf[:, dm0 : dm0 + 1024, 0].reshape(DT, 128, 8, 128).transpose(2, 1, 0, 3)
        ).reshape(8, 128, DT * 128).astype(BF16_NP)
        wk = np.ascontiguousarray(
            Wf[:, dm0 : dm0 + 1024, 2].reshape(DT, 128, 8, 128).transpose(2, 1, 0, 3)
        ).reshape(8, 128, DT * 128).astype(BF16_NP)
        wv = np.ascontiguousarray(
            Wf[:, dm0 : dm0 + 1024, 1].reshape(DT, 128, 2, 512).transpose(2, 1, 0, 3)
        ).reshape(2, 128, DT * 512).astype(BF16_NP)
        bq = np.ascontiguousarray(
            bf[dm0 : dm0 + 1024, 0].reshape(8, 128).T
        ).astype(np.float32)
        bk = np.ascontiguousarray(
            bf[dm0 : dm0 + 1024, 2].reshape(8, 128).T
        ).astype(np.float32)
        bv = bf[dm0 : dm0 + 1024, 1].reshape(2, 4, 128)
        bvq = np.zeros((2, 128, 520), dtype=BF16_NP)
        for g in range(2):
            for hq in range(4):
                bvq[g, :, hq * 130 : hq * 130 + 128] = bv[g, hq][None, :].astype(
                    BF16_NP
                )
                bvq[g, :, hq * 130 + 128] = BF16_NP(1.0)
        in_maps.append(
            {
                "xt": xt, "wq": wq, "wk": wk, "wv": wv,
                "bq": bq, "bk": bk, "bvq": bvq, "mask": mask,
            }
        )
    return in_maps


def gather_out(results):
    out = np.empty((B, S, D), dtype=np.float32)
    for c in range(N_CORES):
        bcore, hg = c // 2, c % 2
        out[bcore][:, hg * 1024 : hg * 1024 + 1024] = results[c]["out"]
    return out


_CACHED_NC = None


def kernel(hidden_states, W, b):
    global _CACHED_NC
    if _CACHED_NC is None:
        _CACHED_NC = build_nc()
    in_maps = make_in_maps(hidden_states, W, b)
    res = run_bass_kernel_spmd(_CACHED_NC, in_maps, core_ids=list(range(N_CORES)))
    return gather_out(res.results)



# revision 5
# speedup vs baseline: 1.1480x; 1.1480x over previous
"""Bloom self-attention (fused QKV + causal softmax attention) on 8 TRN2 cores.

Sharding: core c handles batch b=c//2 and head-group hg=c%2 (8 of 16 heads).
Each core computes QKV projection for its columns of W, then causal attention
for its 8 heads, writing out[s, 1024] (fp32). Host transposes/casts/slices
inputs and gathers outputs.

QKV projection runs in fp8 (e4m3) DoubleRow mode with a 3-term residual
decomposition: X = Xh + Xl, W = Wh + Wl (each fp8 hi + fp8 residual of the
fp32 value), and X@W ~= Xh@Wh + Xh@Wl + Xl@Wh. DoubleRow contracts two
128-deep subtiles per instruction at 0.5 cycles/row, so the 3 products cost
0.75x the bf16 equivalent while keeping ~bf16 accuracy:
  instr1[d]: lhsT=(Wh_d,Wl_d) contiguous pair, rhs=Xh_d broadcast (stride-0)
             -> Wh_d.T Xh_d + Wl_d.T Xh_d
  instr2[d,d+1]: lhsT=(Wh_d,Wh_d+1) stride-2, rhs=(Xl_d,Xl_d+1) stride-2
             -> Wh_d.T Xl_d + Wh_d+1.T Xl_d+1
W is pre-scaled by 32 on host so its entries (~N(0, 1/2048)) sit in fp8's
normal range; the 32x on q,k is folded into the exp scale (/32/32), and the
32x on v is folded into the rowsum-normalization by making the interleaved
"ones" columns 32.0 (biases are pre-scaled by 32 to match).

Layout notes (per core, on device):
  xt8  [128,32,2048] fp8  : X_b^T d-subtiles, (hi_d, lo_d) interleaved
  wq/wk[8,128,32,128] fp8 : per head, (hi_d, lo_d) interleaved W subtiles
  wv   [2,128,32,512] fp8 : per head-quad, 4 heads' v columns batched
  bq/bk[128,8]        f32 : per-partition bias columns per head (x32)
  bvq  [2,128,520]   bf16 : v-bias rows (x32) + interleaved 32.0 columns
                            (130-stride: per quad-head 128 v cols, a 32.0 col,
                             a zero pad col) -> attn@[v|32] yields 32*rowsums
  mask [128,896]     bf16 : causal staircase; slice [:, 384-r:384-r+512] is
                            the 0/1 mask for a diagonal block at offset r
  qT/kT[128,2048]    bf16 : hold 32*q, 32*k

The emission order software-pipelines PE-dense work (QKV/V matmul chunks)
against ACT-bound attention chunks of the previous head, so the Tile
scheduler can fill exp-wait PE gaps with projection matmuls.
"""

import math
from contextlib import ExitStack

import numpy as np
import ml_dtypes

import concourse.mybir as mybir
import concourse.tile as tile
from concourse import bacc
from concourse.bass_utils import run_bass_kernel_spmd

B, S, D = 4, 2048, 2048
H, HD = 16, 128
N_CORES = 8
DT = D // 128   # 16 d-subtiles
NQB = S // 512  # 4 q-blocks
SCALE = 1.0 / math.sqrt(HD)
WS = 32.0       # host-side W pre-scale

BF16 = mybir.dt.bfloat16
F32 = mybir.dt.float32
FP8 = mybir.dt.float8e4
DR = mybir.MatmulPerfMode.DoubleRow
BF16_NP = ml_dtypes.bfloat16
FP8_NP = ml_dtypes.float8_e4m3fn


def build_nc(repeat: int = 1):
    nc = bacc.Bacc(
        "TRN2",
        target_bir_lowering=False,
        debug=False,
        enable_asserts=False,
        num_devices=N_CORES,
    )
    xt_d = nc.dram_tensor("xt", [2 * DT, 128, S], FP8, kind="ExternalInput")
    wq_d = nc.dram_tensor("wq", [8, 128, 2 * DT * 128], FP8, kind="ExternalInput")
    wk_d = nc.dram_tensor("wk", [8, 128, 2 * DT * 128], FP8, kind="ExternalInput")
    wv_d = nc.dram_tensor("wv", [2, 128, 2 * DT * 512], FP8, kind="ExternalInput")
    bq_d = nc.dram_tensor("bq", [128, 8], F32, kind="ExternalInput")
    bk_d = nc.dram_tensor("bk", [128, 8], F32, kind="ExternalInput")
    bvq_d = nc.dram_tensor("bvq", [2, 128, 520], BF16, kind="ExternalInput")
    mask_d = nc.dram_tensor("mask", [128, 896], BF16, kind="ExternalInput")
    out_d = nc.dram_tensor("out", [S, 1024], F32, kind="ExternalOutput")

    with ExitStack() as ctx:
        tc = ctx.enter_context(tile.TileContext(nc))
        singles = ctx.enter_context(tc.tile_pool(name="singles", bufs=1))
        wqk_pool = ctx.enter_context(tc.tile_pool(name="wqk", bufs=2))
        wv_pool = ctx.enter_context(tc.tile_pool(name="wvp", bufs=2))
        qk_pool = ctx.enter_context(tc.tile_pool(name="qk", bufs=2))
        v4_pool = ctx.enter_context(tc.tile_pool(name="v4", bufs=2))
        p_pool = ctx.enter_context(tc.tile_pool(name="pp", bufs=24))
        o_pool = ctx.enter_context(tc.tile_pool(name="op", bufs=8))
        r_pool = ctx.enter_context(tc.tile_pool(name="rp", bufs=8))
        ps_big = ctx.enter_context(tc.tile_pool(name="ps_big", bufs=7, space="PSUM"))
        ps_out = ctx.enter_context(tc.tile_pool(name="ps_out", bufs=1, space="PSUM"))

        # ---- resident constants (loaded once) ----
        # Interleave wv(0) subtile DMAs with the xt subtile DMAs: the first
        # (d-outer) v-chunk consumes exactly wv[d-pair]+xt[d-pair] per step,
        # so the PE ramp starts as soon as the first pairs land.
        wv_first = wv_pool.tile([128, 2 * DT, 512], FP8, tag="wv")
        xt = singles.tile([128, 2 * DT, S], FP8, tag="xt")
        for k in range(2 * DT):
            nc.sync.dma_start(
                out=wv_first[:, k, :],
                in_=wv_d.ap()[0].rearrange("p (k n) -> p k n", k=2 * DT)[:, k, :],
            )
            nc.sync.dma_start(out=xt[:, k, :], in_=xt_d.ap()[k])
        mask = singles.tile([128, 896], BF16, tag="mask")
        nc.sync.dma_start(out=mask[:], in_=mask_d.ap())
        bvq = singles.tile([128, 2 * 520], BF16, tag="bvq")
        for g in range(2):
            nc.sync.dma_start(
                out=bvq[:, g * 520 : (g + 1) * 520], in_=bvq_d.ap()[g, :, :]
            )
        bq = singles.tile([128, 8], F32, tag="bq")
        nc.sync.dma_start(out=bq[:], in_=bq_d.ap())
        bk = singles.tile([128, 8], F32, tag="bk")
        nc.sync.dma_start(out=bk[:], in_=bk_d.ap())
        # prewarm the ACT exp table set (~2.7us PSEUDO_LOAD on first Exp)
        # while the startup DMAs run, instead of inside the first attention
        # chain
        warm = singles.tile([128, 1], F32, tag="warm")
        nc.vector.memset(warm[:], 0.0)
        nc.scalar.activation(warm[:], warm[:], mybir.ActivationFunctionType.Exp)

        def resid_mms(psx, w8, xs0, n_s):
            """Emit the 24 DoubleRow matmuls for psx[128,n_s] = (X@W).T chunk
            over the full D contraction, for s-columns [xs0, xs0+n_s).
            w8: [128, 2*DT, M] tile with (hi_d, lo_d) interleaved subtiles."""
            wvw = w8[:].rearrange("p (d two) m -> p two d m", two=2)
            xv = xt[:].rearrange("p (d two) s -> p two d s", two=2)
            for d in range(0, DT, 2):
                for dd in (d, d + 1):
                    nc.tensor.matmul(
                        psx,
                        lhsT=w8[:, 2 * dd : 2 * dd + 2, :],
                        rhs=xt[:, 2 * dd, xs0 : xs0 + n_s]
                        .unsqueeze(1)
                        .to_broadcast([128, 2, n_s]),
                        start=(dd == 0),
                        stop=False,
                        perf_mode=DR,
                    )
                nc.tensor.matmul(
                    psx,
                    lhsT=wvw[:, 0, d : d + 2, :],
                    rhs=xv[:, 1, d : d + 2, xs0 : xs0 + n_s],
                    start=False,
                    stop=(d == DT - 2),
                    perf_mode=DR,
                )

        for _rep in range(repeat):
            # per-rep state: tiles keyed by quad / head
            v4s = {}     # g -> [16 v4 tiles]
            wv_gs = {}   # g -> wv tile
            qks = {}     # h -> (qT, kT, wq_h, wk_h)

            def v_start(g):
                if g == 0 and _rep == 0:
                    wv_g = wv_first
                else:
                    wv_g = wv_pool.tile([128, 2 * DT, 512], FP8, tag="wv")
                    nc.sync.dma_start(
                        out=wv_g[:].rearrange("p k n -> p (k n)"),
                        in_=wv_d.ap()[g],
                    )
                wv_gs[g] = wv_g
                v4s[g] = []

            def v_chunk(g, sts):
                """v4[st] = X @ Wv_quad + bv (+ interleaved 32.0 cols).

                Here X is the stationary side (psum partitions = s rows) and
                Wv the moving side, so the residual pairing flips: instr1
                lhsT=(Xh_d,Xl_d) vs broadcast Wh_d; instr2 strided Xh vs
                strided Wl. d-outer over the st group so each xt subtile is
                consumed as soon as its DMA lands (matters for the startup
                ramp)."""
                wv_g = wv_gs[g]
                wvv = wv_g[:].rearrange("p (d two) n -> p two d n", two=2)
                xv = xt[:].rearrange("p (d two) s -> p two d s", two=2)
                sts = list(sts)
                psvs = []
                for _st in sts:
                    psv = ps_big.tile([128, 512], F32, tag="ps_big")
                    psvs.append(psv)
                for d in range(0, DT, 2):
                    for dd in (d, d + 1):
                        for st, psv in zip(sts, psvs):
                            nc.tensor.matmul(
                                psv[:],
                                lhsT=xt[:, 2 * dd : 2 * dd + 2,
                                        st * 128 : (st + 1) * 128],
                                rhs=wvv[:, 0, dd, :]
                                .unsqueeze(1)
                                .to_broadcast([128, 2, 512]),
                                start=(dd == 0),
                                stop=False,
                                perf_mode=DR,
                            )
                    for st, psv in zip(sts, psvs):
                        nc.tensor.matmul(
                            psv[:],
                            lhsT=xv[:, 0, d : d + 2, st * 128 : (st + 1) * 128],
                            rhs=wvv[:, 1, d : d + 2, :],
                            start=False,
                            stop=(d == DT - 2),
                            perf_mode=DR,
                        )
                for st, psv in zip(sts, psvs):
                    v4t = v4_pool.tile([128, 520], BF16, tag=f"v4_{st}")
                    nc.vector.tensor_copy(v4t[:], bvq[:, g * 520 : (g + 1) * 520])
                    dst = v4t[:].rearrange("p (q c) -> p q c", q=4)[:, :, 0:128]
                    src = psv[:].rearrange("p (q c) -> p q c", q=4)
                    nc.vector.tensor_add(dst, dst, src)
                    v4s[g].append(v4t)

            def qkv_start(h):
                wq_h = wqk_pool.tile([128, 2 * DT, 128], FP8, tag="wq")
                nc.sync.dma_start(
                    out=wq_h[:].rearrange("p k m -> p (k m)"), in_=wq_d.ap()[h]
                )
                wk_h = wqk_pool.tile([128, 2 * DT, 128], FP8, tag="wk")
                nc.sync.dma_start(
                    out=wk_h[:].rearrange("p k m -> p (k m)"), in_=wk_d.ap()[h]
                )
                qT = qk_pool.tile([128, S], BF16, tag="qT")
                kT = qk_pool.tile([128, S], BF16, tag="kT")
                qks[h] = (qT, kT, wq_h, wk_h)

            def qkv_chunk(h, sb):
                """qT/kT columns for s-block sb of head h."""
                qT, kT, wq_h, wk_h = qks[h]
                for w_h, dest, bias in ((wq_h, qT, bq), (wk_h, kT, bk)):
                    psx = ps_big.tile([128, 512], F32, tag="ps_big")
                    resid_mms(psx[:], w_h, sb * 512, 512)
                    nc.vector.tensor_scalar_add(
                        dest[:, sb * 512 : (sb + 1) * 512], psx[:], bias[:, h : h + 1]
                    )

            attn_ps = {}  # (h, qb) -> [(p_tile, off)]

            def attn_scores(h, qb, lo=0, hi=None):
                """Scores + exp (+causal mask) for q-block qb of head h.

                Diagonal k-tiles are trimmed to their live width: tile kt
                covers q_local in [off, 512) with off = max(kt*128-qb*512, 0).
                """
                qT, kT = qks[h][0], qks[h][1]
                n_kt = 4 * qb + 4
                if hi is None:
                    hi = n_kt
                ps = attn_ps.setdefault((h, qb), [])
                for kt in range(lo, hi):
                    r = kt * 128 - qb * 512
                    off = max(r, 0)
                    nw = 512 - off
                    pss = ps_big.tile([128, 512], F32, tag="ps_big")
                    nc.tensor.matmul(
                        pss[:, 0:nw],
                        lhsT=kT[:, kt * 128 : (kt + 1) * 128],
                        rhs=qT[:, qb * 512 + off : (qb + 1) * 512],
                        start=True,
                        stop=True,
                    )
                    p_sb = p_pool.tile([128, 512], BF16, tag="p")
                    nc.scalar.activation(
                        p_sb[:, 0:nw], pss[:, 0:nw],
                        mybir.ActivationFunctionType.Exp, scale=SCALE / (WS * WS),
                    )
                    if r >= 0:  # diagonal block: apply causal 0/1 mask
                        nc.vector.tensor_mul(
                            p_sb[:, 0:nw], p_sb[:, 0:nw], mask[:, 384 : 384 + nw]
                        )
                    ps.append((p_sb, off))

            def attn_out(h, qb):
                """attn @ [v|32], normalize, and store, for q-block qb."""
                g, hq = h // 4, h % 4
                v4 = v4s[g]
                ps = attn_ps.pop((h, qb))
                for j in range(4):
                    poj = ps_out.tile([128, 129], F32, tag="po")
                    last_kt = 4 * qb + j  # causality: kt*128 <= qb*512 + j*128
                    for kt in range(last_kt + 1):
                        p_sb, off = ps[kt]
                        nc.tensor.matmul(
                            poj[:],
                            lhsT=p_sb[:, j * 128 - off : j * 128 - off + 128],
                            rhs=v4[kt][:, hq * 130 : hq * 130 + 129],
                            start=(kt == 0),
                            stop=(kt == last_kt),
                        )
                    recip = r_pool.tile([128, 1], F32, tag="recip")
                    nc.vector.reciprocal(recip[:], poj[:, 128:129])
                    o_sb = o_pool.tile([128, 128], F32, tag="o")
                    nc.vector.tensor_scalar_mul(o_sb[:], poj[:, 0:128], recip[:])
                    nc.sync.dma_start(
                        out=out_d.ap()[
                            qb * 512 + j * 128 : qb * 512 + (j + 1) * 128,
                            h * 128 : (h + 1) * 128,
                        ],
                        in_=o_sb[:],
                    )

            # ---- software-pipelined emission ----
            # Each step pairs a PE-dense item (4 chunks) with the attention of
            # an already-projected head: scores(qb) -> pe chunk -> out(qb), so
            # exps for qb run on ACT while PE does projection matmuls.
            def v_item(g):
                v_start(g)
                return [lambda i=i: v_chunk(g, range(4 * i, 4 * i + 4))
                        for i in range(4)]

            def qkv_item(h):
                qkv_start(h)
                return [lambda sb=sb: qkv_chunk(h, sb) for sb in range(NQB)]

            steps = [
                (lambda: v_item(0), None),
                (lambda: qkv_item(0), None),
                (lambda: qkv_item(1), 0),
                (lambda: qkv_item(2), 1),
                (lambda: qkv_item(3), 2),
                (lambda: qkv_item(4), 3),
                (lambda: v_item(1), None),
                (lambda: qkv_item(5), 4),
                (lambda: qkv_item(6), 5),
                (lambda: qkv_item(7), 6),
                (None, 7),
            ]
            for pe_item, h_attn in steps:
                pe_chunks = pe_item() if pe_item is not None else [None] * NQB
                _lo = {0: 0, 1: 2, 2: 4, 3: 5}
                for i in range(NQB):
                    if h_attn is not None:
                        attn_scores(h_attn, i, lo=_lo[i])
                    if pe_chunks[i] is not None:
                        pe_chunks[i]()
                    if h_attn is not None:
                        if i + 1 < NQB and _lo[i + 1] > 0:
                            attn_scores(h_attn, i + 1, lo=0, hi=_lo[i + 1])
                        attn_out(h_attn, i)
    nc.compile()
    return nc


def _fp8_split(a):
    """fp32 array -> (hi, lo) fp8 e4m3 with hi + lo ~= a."""
    hi = a.astype(FP8_NP)
    lo = (a - hi.astype(np.float32)).astype(FP8_NP)
    return hi, lo


def make_in_maps(hidden_states, W, b):
    """Host-side sharding: slice/transpose/cast inputs per core."""
    X = np.asarray(hidden_states, dtype=np.float32)
    Wf = np.asarray(W, dtype=np.float32).reshape(D, D, 3) * WS
    bf = np.asarray(b, dtype=np.float32).reshape(D, 3) * WS

    # causal staircase mask: mask[p, c] = 1 if c >= p + 384
    cols = np.arange(896)[None, :]
    rows = np.arange(128)[:, None]
    mask = (cols >= rows + 384).astype(BF16_NP)

    def interleave(hi, lo, m):
        # [DT, 128, m] pair -> [128, 2*DT, m] with (hi_d, lo_d) subtiles
        st = np.stack([hi, lo], axis=1)  # [DT, 2, 128, m]
        return np.ascontiguousarray(st.transpose(2, 0, 1, 3)).reshape(
            128, 2 * DT, m
        )

    in_maps = []
    for c in range(N_CORES):
        bcore, hg = c // 2, c % 2
        dm0 = hg * 1024
        xtf = np.ascontiguousarray(X[bcore].T).reshape(DT, 128, S)
        xh, xl = _fp8_split(xtf)
        # dram layout [2*DT, 128, S] with order (hi_0, lo_0, hi_1, ...)
        xt8 = np.stack([xh, xl], axis=1).reshape(2 * DT, 128, S)

        def w_head(cols3, t, m):
            # cols3: column range within this core's 1024; t: 0=q,1=v,2=k
            wf = Wf[:, dm0 + cols3[0] : dm0 + cols3[1], t].reshape(DT, 128, m)
            hi, lo = _fp8_split(wf)
            return interleave(hi, lo, m).reshape(128, 2 * DT * m)

        wq = np.stack([w_head((h * 128, (h + 1) * 128), 0, 128) for h in range(8)])
        wk = np.stack([w_head((h * 128, (h + 1) * 128), 2, 128) for h in range(8)])
        wv = np.stack([w_head((g * 512, (g + 1) * 512), 1, 512) for g in range(2)])
        bq = np.ascontiguousarray(
            bf[dm0 : dm0 + 1024, 0].reshape(8, 128).T
        ).astype(np.float32)
        bk = np.ascontiguousarray(
            bf[dm0 : dm0 + 1024, 2].reshape(8, 128).T
        ).astype(np.float32)
        bv = bf[dm0 : dm0 + 1024, 1].reshape(2, 4, 128)
        bvq = np.zeros((2, 128, 520), dtype=BF16_NP)
        for g in range(2):
            for hq in range(4):
                bvq[g, :, hq * 130 : hq * 130 + 128] = bv[g, hq][None, :].astype(
                    BF16_NP
                )
                bvq[g, :, hq * 130 + 128] = BF16_NP(WS)
        in_maps.append(
            {
                "xt": xt8, "wq": wq, "wk": wk, "wv": wv,
                "bq": bq, "bk": bk, "bvq": bvq, "mask": mask,
            }
        )
    return in_maps


def gather_out(results):
    out = np.empty((B, S, D), dtype=np.float32)
    for c in range(N_CORES):
        bcore, hg = c // 2, c % 2
        out[bcore][:, hg * 1024 : hg * 1024 + 1024] = results[c]["out"]
    return out


_CACHED_NC = None


def kernel(hidden_states, W, b):
    global _CACHED_NC
    if _CACHED_NC is None:
        _CACHED_NC = build_nc()
    in_maps = make_in_maps(hidden_states, W, b)
    res = run_bass_kernel_spmd(_CACHED_NC, in_maps, core_ids=list(range(N_CORES)))
    return gather_out(res.results)


# revision 6
# speedup vs baseline: 1.1803x; 1.0282x over previous
"""Bloom self-attention (fused QKV + causal softmax attention) on 8 TRN2 cores.

Sharding: core c handles batch b=c//2 and head-group hg=c%2 (8 of 16 heads).
Each core computes QKV projection for its columns of W, then causal attention
for its 8 heads, writing out[s, 1024] (fp32). Host transposes/casts/slices
inputs and gathers outputs.

QKV projection runs in fp8 (e4m3) DoubleRow mode with a 3-term residual
decomposition: X = Xh + Xl, W = Wh + Wl (each fp8 hi + fp8 residual of the
fp32 value), and X@W ~= Xh@Wh + Xh@Wl + Xl@Wh. DoubleRow contracts two
128-deep subtiles per instruction at 0.5 cycles/row, so the 3 products cost
0.75x the bf16 equivalent while keeping ~bf16 accuracy:
  instr1[d]: lhsT=(Wh_d,Wl_d) contiguous pair, rhs=Xh_d broadcast (stride-0)
             -> Wh_d.T Xh_d + Wl_d.T Xh_d
  instr2[d,d+1]: lhsT=(Wh_d,Wh_d+1) stride-2, rhs=(Xl_d,Xl_d+1) stride-2
             -> Wh_d.T Xl_d + Wh_d+1.T Xl_d+1
W is pre-scaled by 32 on host so its entries (~N(0, 1/2048)) sit in fp8's
normal range; the 32x on q,k is folded into the exp scale (/32/32), and the
32x on v is folded into the rowsum-normalization by making the interleaved
"ones" columns 32.0 (biases are pre-scaled by 32 to match).

DMAs are batched (multi-subtile xt/wv groups, fused q+k weight loads, one
output DMA per (head, q-block) via a j-interleaved dram view) because the
HWDGE descriptor generator is a serial ~630ns/DMA resource that otherwise
starves the PE during the startup ramp.

Layout notes (per core, on device):
  xt8  [128,32,2048] fp8  : X_b^T d-subtiles, (hi_d, lo_d) interleaved
  wqk  [8,128,64,128] fp8 : per head, q then k (hi_d, lo_d) W subtiles
  wv   [2,128,32,512] fp8 : per head-quad, 4 heads' v columns batched
  bqk  [128,16]       f32 : per-partition bias columns per head (x32), q|k
  bvq  [2,128,520]   bf16 : v-bias rows (x32) + interleaved 32.0 columns
                            (130-stride: per quad-head 128 v cols, a 32.0 col,
                             a zero pad col) -> attn@[v|32] yields 32*rowsums
  mask [128,896]     bf16 : causal staircase; slice [:, 384-r:384-r+512] is
                            the 0/1 mask for a diagonal block at offset r
  qT/kT[128,2048]    bf16 : hold 32*q, 32*k

The emission order software-pipelines PE-dense work (QKV/V matmul chunks)
against ACT-bound attention chunks of an already-projected head; the last
three heads run "JIT" (projection chunk sb emitted just before the score
chunks that need it) so no step is left with attention-only PE work.
"""

import math
from contextlib import ExitStack

import numpy as np
import ml_dtypes

import concourse.mybir as mybir
import concourse.tile as tile
from concourse import bacc
from concourse.bass_utils import run_bass_kernel_spmd

B, S, D = 4, 2048, 2048
H, HD = 16, 128
N_CORES = 8
DT = D // 128   # 16 d-subtiles
NQB = S // 512  # 4 q-blocks
SCALE = 1.0 / math.sqrt(HD)
WS = 32.0       # host-side W pre-scale

BF16 = mybir.dt.bfloat16
F32 = mybir.dt.float32
FP8 = mybir.dt.float8e4
DR = mybir.MatmulPerfMode.DoubleRow
BF16_NP = ml_dtypes.bfloat16
FP8_NP = ml_dtypes.float8_e4m3fn


def build_nc(repeat: int = 1):
    nc = bacc.Bacc(
        "TRN2",
        target_bir_lowering=False,
        debug=False,
        enable_asserts=False,
        num_devices=N_CORES,
    )
    xt_d = nc.dram_tensor("xt", [2 * DT, 128, S], FP8, kind="ExternalInput")
    wqk_d = nc.dram_tensor("wqk", [8, 128, 2 * 2 * DT * 128], FP8,
                           kind="ExternalInput")
    wv_d = nc.dram_tensor("wv", [2, 128, 2 * DT * 512], FP8, kind="ExternalInput")
    bqk_d = nc.dram_tensor("bqk", [128, 16], F32, kind="ExternalInput")
    bvq_d = nc.dram_tensor("bvq", [2, 128, 520], BF16, kind="ExternalInput")
    mask_d = nc.dram_tensor("mask", [128, 896], BF16, kind="ExternalInput")
    out_d = nc.dram_tensor("out", [S, 1024], F32, kind="ExternalOutput")

    with ExitStack() as ctx:
        tc = ctx.enter_context(tile.TileContext(nc))
        singles = ctx.enter_context(tc.tile_pool(name="singles", bufs=1))
        wqk_pool = ctx.enter_context(tc.tile_pool(name="wqk", bufs=2))
        wv_pool = ctx.enter_context(tc.tile_pool(name="wvp", bufs=2))
        qk_pool = ctx.enter_context(tc.tile_pool(name="qk", bufs=2))
        v4_pool = ctx.enter_context(tc.tile_pool(name="v4", bufs=2))
        p_pool = ctx.enter_context(tc.tile_pool(name="pp", bufs=24))
        o_pool = ctx.enter_context(tc.tile_pool(name="op", bufs=3))
        r_pool = ctx.enter_context(tc.tile_pool(name="rp", bufs=8))
        ps_big = ctx.enter_context(tc.tile_pool(name="ps_big", bufs=7, space="PSUM"))
        ps_out = ctx.enter_context(tc.tile_pool(name="ps_out", bufs=1, space="PSUM"))

        # ---- resident constants (loaded once) ----
        # Batched startup DMAs: xt in 8 groups of 4 subtiles (2 d-pairs),
        # wv(0) in 4 groups of 8 subtiles, interleaved so the d-outer first
        # v-chunk can start as soon as the first groups land. One DMA costs
        # ~630ns of serial HWDGE regardless of size, so bigger is better as
        # long as the first group lands quickly.
        wv_first = wv_pool.tile([128, 2 * DT, 512], FP8, tag="wv")
        xt = singles.tile([128, 2 * DT, S], FP8, tag="xt")
        xt_dv = xt_d.ap().rearrange("(g k) p s -> p g k s", k=4)
        wv_dv = wv_d.ap()[0].rearrange("p (g k n) -> p g k n", g=4, n=512)
        for g in range(8):
            nc.sync.dma_start(out=xt[:, 4 * g : 4 * g + 4, :], in_=xt_dv[:, g])
            if g % 2 == 0:
                nc.sync.dma_start(
                    out=wv_first[:, 8 * (g // 2) : 8 * (g // 2) + 8, :],
                    in_=wv_dv[:, g // 2],
                )
        mask = singles.tile([128, 896], BF16, tag="mask")
        nc.sync.dma_start(out=mask[:], in_=mask_d.ap())
        bvq = singles.tile([128, 2, 520], BF16, tag="bvq")
        nc.sync.dma_start(out=bvq[:], in_=bvq_d.ap().rearrange("g p c -> p g c"))
        bqk = singles.tile([128, 16], F32, tag="bqk")
        nc.sync.dma_start(out=bqk[:], in_=bqk_d.ap())
        # prewarm the ACT exp table set (~2.7us PSEUDO_LOAD on first Exp)
        # while the startup DMAs run, instead of inside the first attention
        # chain
        warm = singles.tile([128, 1], F32, tag="warm")
        nc.vector.memset(warm[:], 0.0)
        nc.scalar.activation(warm[:], warm[:], mybir.ActivationFunctionType.Exp)

        def resid_mms(psx, w8, xs0, n_s):
            """Emit the 24 DoubleRow matmuls for psx[128,n_s] = (X@W).T chunk
            over the full D contraction, for s-columns [xs0, xs0+n_s).
            w8: [128, 2*DT, M] AP with (hi_d, lo_d) interleaved subtiles."""
            wvw = w8.rearrange("p (d two) m -> p two d m", two=2)
            xv = xt[:].rearrange("p (d two) s -> p two d s", two=2)
            for d in range(0, DT, 2):
                for dd in (d, d + 1):
                    nc.tensor.matmul(
                        psx,
                        lhsT=w8[:, 2 * dd : 2 * dd + 2, :],
                        rhs=xt[:, 2 * dd, xs0 : xs0 + n_s]
                        .unsqueeze(1)
                        .to_broadcast([128, 2, n_s]),
                        start=(dd == 0),
                        stop=False,
                        perf_mode=DR,
                    )
                nc.tensor.matmul(
                    psx,
                    lhsT=wvw[:, 0, d : d + 2, :],
                    rhs=xv[:, 1, d : d + 2, xs0 : xs0 + n_s],
                    start=False,
                    stop=(d == DT - 2),
                    perf_mode=DR,
                )

        for _rep in range(repeat):
            # per-rep state: tiles keyed by quad / head
            v4s = {}     # g -> [16 v4 tiles]
            wv_gs = {}   # g -> wv tile
            qks = {}     # h -> (qT, kT, wqk_h)

            def v_start(g):
                if g == 0 and _rep == 0:
                    wv_g = wv_first
                else:
                    wv_g = wv_pool.tile([128, 2 * DT, 512], FP8, tag="wv")
                    nc.sync.dma_start(
                        out=wv_g[:].rearrange("p k n -> p (k n)"),
                        in_=wv_d.ap()[g],
                    )
                wv_gs[g] = wv_g
                v4s[g] = []

            def v_chunk(g, sts):
                """v4[st] = X @ Wv_quad + bv (+ interleaved 32.0 cols).

                Here X is the stationary side (psum partitions = s rows) and
                Wv the moving side, so the residual pairing flips: instr1
                lhsT=(Xh_d,Xl_d) vs broadcast Wh_d; instr2 strided Xh vs
                strided Wl. d-outer over the st group so each xt subtile is
                consumed as soon as its DMA lands (matters for the startup
                ramp)."""
                wv_g = wv_gs[g]
                wvv = wv_g[:].rearrange("p (d two) n -> p two d n", two=2)
                xv = xt[:].rearrange("p (d two) s -> p two d s", two=2)
                sts = list(sts)
                psvs = []
                for _st in sts:
                    psv = ps_big.tile([128, 512], F32, tag="ps_big")
                    psvs.append(psv)
                for d in range(0, DT, 2):
                    for dd in (d, d + 1):
                        for st, psv in zip(sts, psvs):
                            nc.tensor.matmul(
                                psv[:],
                                lhsT=xt[:, 2 * dd : 2 * dd + 2,
                                        st * 128 : (st + 1) * 128],
                                rhs=wvv[:, 0, dd, :]
                                .unsqueeze(1)
                                .to_broadcast([128, 2, 512]),
                                start=(dd == 0),
                                stop=False,
                                perf_mode=DR,
                            )
                    for st, psv in zip(sts, psvs):
                        nc.tensor.matmul(
                            psv[:],
                            lhsT=xv[:, 0, d : d + 2, st * 128 : (st + 1) * 128],
                            rhs=wvv[:, 1, d : d + 2, :],
                            start=False,
                            stop=(d == DT - 2),
                            perf_mode=DR,
                        )
                for st, psv in zip(sts, psvs):
                    v4t = v4_pool.tile([128, 520], BF16, tag=f"v4_{st}")
                    nc.vector.tensor_copy(v4t[:], bvq[:, g, :])
                    dst = v4t[:].rearrange("p (q c) -> p q c", q=4)[:, :, 0:128]
                    src = psv[:].rearrange("p (q c) -> p q c", q=4)
                    nc.vector.tensor_add(dst, dst, src)
                    v4s[g].append(v4t)

            def qkv_start(h):
                wqk_h = wqk_pool.tile([128, 4 * DT, 128], FP8, tag="wqk")
                nc.sync.dma_start(
                    out=wqk_h[:].rearrange("p k m -> p (k m)"), in_=wqk_d.ap()[h]
                )
                qT = qk_pool.tile([128, S], BF16, tag="qT")
                kT = qk_pool.tile([128, S], BF16, tag="kT")
                qks[h] = (qT, kT, wqk_h)

            def qkv_chunk(h, sb):
                """qT/kT columns for s-block sb of head h."""
                qT, kT, wqk_h = qks[h]
                for idx, dest in ((0, qT), (1, kT)):
                    psx = ps_big.tile([128, 512], F32, tag="ps_big")
                    resid_mms(
                        psx[:],
                        wqk_h[:, 2 * DT * idx : 2 * DT * (idx + 1), :],
                        sb * 512,
                        512,
                    )
                    nc.vector.tensor_scalar_add(
                        dest[:, sb * 512 : (sb + 1) * 512], psx[:],
                        bqk[:, 8 * idx + h : 8 * idx + h + 1],
                    )

            attn_ps = {}  # (h, qb) -> [(p_tile, off)]

            def attn_scores(h, qb, lo=0, hi=None):
                """Scores + exp (+causal mask) for q-block qb of head h.

                Diagonal k-tiles are trimmed to their live width: tile kt
                covers q_local in [off, 512) with off = max(kt*128-qb*512, 0).
                """
                qT, kT = qks[h][0], qks[h][1]
                n_kt = 4 * qb + 4
                if hi is None:
                    hi = n_kt
                ps = attn_ps.setdefault((h, qb), [])
                for kt in range(lo, hi):
                    r = kt * 128 - qb * 512
                    off = max(r, 0)
                    nw = 512 - off
                    pss = ps_big.tile([128, 512], F32, tag="ps_big")
                    nc.tensor.matmul(
                        pss[:, 0:nw],
                        lhsT=kT[:, kt * 128 : (kt + 1) * 128],
                        rhs=qT[:, qb * 512 + off : (qb + 1) * 512],
                        start=True,
                        stop=True,
                    )
                    p_sb = p_pool.tile([128, 512], BF16, tag="p")
                    nc.scalar.activation(
                        p_sb[:, 0:nw], pss[:, 0:nw],
                        mybir.ActivationFunctionType.Exp, scale=SCALE / (WS * WS),
                    )
                    if r >= 0:  # diagonal block: apply causal 0/1 mask
                        nc.vector.tensor_mul(
                            p_sb[:, 0:nw], p_sb[:, 0:nw], mask[:, 384 : 384 + nw]
                        )
                    ps.append((p_sb, off))

            def attn_out(h, qb):
                """attn @ [v|32], normalize, and store, for q-block qb."""
                g, hq = h // 4, h % 4
                v4 = v4s[g]
                ps = attn_ps.pop((h, qb))
                o4 = o_pool.tile([128, 512], F32, tag="o")
                for j in range(4):
                    poj = ps_out.tile([128, 129], F32, tag="po")
                    last_kt = 4 * qb + j  # causality: kt*128 <= qb*512 + j*128
                    for kt in range(last_kt + 1):
                        p_sb, off = ps[kt]
                        nc.tensor.matmul(
                            poj[:],
                            lhsT=p_sb[:, j * 128 - off : j * 128 - off + 128],
                            rhs=v4[kt][:, hq * 130 : hq * 130 + 129],
                            start=(kt == 0),
                            stop=(kt == last_kt),
                        )
                    recip = r_pool.tile([128, 1], F32, tag="recip")
                    nc.vector.reciprocal(recip[:], poj[:, 128:129])
                    nc.vector.tensor_scalar_mul(
                        o4[:, j * 128 : (j + 1) * 128], poj[:, 0:128], recip[:]
                    )
                # one DMA per (h, qb): dram view [512,128] -> [128 p, 4 j, 128]
                nc.sync.dma_start(
                    out=out_d.ap()[
                        qb * 512 : (qb + 1) * 512, h * 128 : (h + 1) * 128
                    ].rearrange("(j p) c -> p j c", j=4),
                    in_=o4[:].rearrange("p (j c) -> p j c", j=4),
                )

            # ---- software-pipelined emission ----
            # (pe_item, h_attn, jit): pe_item's 4 chunks are interleaved with
            # the attention of head h_attn. jit=False: h_attn was projected in
            # an earlier step; scores(qb) -> pe chunk -> out(qb) so exps run
            # on ACT while PE does projection matmuls. jit=True: pe_item IS
            # qkv_item(h_attn); chunk sb runs just before the scores that
            # need it (shift-by-one), so the last heads still overlap exp
            # with their own projection matmuls.
            def v_item(g):
                v_start(g)
                return [lambda i=i: v_chunk(g, range(4 * i, 4 * i + 4))
                        for i in range(4)]

            def qkv_item(h):
                qkv_start(h)
                return [lambda sb=sb: qkv_chunk(h, sb) for sb in range(NQB)]

            steps = [
                (lambda: v_item(0), None, False),
                (lambda: qkv_item(0), None, False),
                (lambda: qkv_item(1), 0, False),
                (lambda: qkv_item(2), 1, False),
                (lambda: qkv_item(3), 2, False),
                (lambda: qkv_item(4), 3, False),
                (lambda: v_item(1), 4, False),
                (lambda: qkv_item(5), 5, True),
                (lambda: qkv_item(6), 6, True),
                (lambda: qkv_item(7), 7, True),
            ]
            _lo = {0: 0, 1: 2, 2: 4, 3: 5}
            for pe_item, h_attn, jit in steps:
                pe_chunks = pe_item() if pe_item is not None else [None] * NQB
                if jit:
                    pe_chunks[0]()
                for i in range(NQB):
                    if h_attn is not None:
                        attn_scores(h_attn, i, lo=_lo[i])
                    if not jit and pe_chunks[i] is not None:
                        pe_chunks[i]()
                    if jit and i + 1 < NQB:
                        pe_chunks[i + 1]()
                    if h_attn is not None:
                        if i + 1 < NQB and _lo[i + 1] > 0:
                            attn_scores(h_attn, i + 1, lo=0, hi=_lo[i + 1])
                        attn_out(h_attn, i)
    nc.compile()
    return nc


def _fp8_split(a):
    """fp32 array -> (hi, lo) fp8 e4m3 with hi + lo ~= a."""
    hi = a.astype(FP8_NP)
    lo = (a - hi.astype(np.float32)).astype(FP8_NP)
    return hi, lo


def make_in_maps(hidden_states, W, b):
    """Host-side sharding: slice/transpose/cast inputs per core."""
    X = np.asarray(hidden_states, dtype=np.float32)
    Wf = np.asarray(W, dtype=np.float32).reshape(D, D, 3) * WS
    bf = np.asarray(b, dtype=np.float32).reshape(D, 3) * WS

    # causal staircase mask: mask[p, c] = 1 if c >= p + 384
    cols = np.arange(896)[None, :]
    rows = np.arange(128)[:, None]
    mask = (cols >= rows + 384).astype(BF16_NP)

    def interleave(hi, lo, m):
        # [DT, 128, m] pair -> [128, 2*DT, m] with (hi_d, lo_d) subtiles
        st = np.stack([hi, lo], axis=1)  # [DT, 2, 128, m]
        return np.ascontiguousarray(st.transpose(2, 0, 1, 3)).reshape(
            128, 2 * DT, m
        )

    in_maps = []
    for c in range(N_CORES):
        bcore, hg = c // 2, c % 2
        dm0 = hg * 1024
        xtf = np.ascontiguousarray(X[bcore].T).reshape(DT, 128, S)
        xh, xl = _fp8_split(xtf)
        # dram layout [2*DT, 128, S] with order (hi_0, lo_0, hi_1, ...)
        xt8 = np.stack([xh, xl], axis=1).reshape(2 * DT, 128, S)

        def w_cols(c0, c1, t, m):
            # cols [c0,c1) within this core's 1024; t: 0=q,1=v,2=k
            wf = Wf[:, dm0 + c0 : dm0 + c1, t].reshape(DT, 128, m)
            hi, lo = _fp8_split(wf)
            return interleave(hi, lo, m)

        wqk = np.stack(
            [
                np.concatenate(
                    [
                        w_cols(h * 128, (h + 1) * 128, 0, 128),
                        w_cols(h * 128, (h + 1) * 128, 2, 128),
                    ],
                    axis=1,
                ).reshape(128, 4 * DT * 128)
                for h in range(8)
            ]
        )
        wv = np.stack(
            [
                w_cols(g * 512, (g + 1) * 512, 1, 512).reshape(128, 2 * DT * 512)
                for g in range(2)
            ]
        )
        bqk = np.concatenate(
            [
                np.ascontiguousarray(bf[dm0 : dm0 + 1024, 0].reshape(8, 128).T),
                np.ascontiguousarray(bf[dm0 : dm0 + 1024, 2].reshape(8, 128).T),
            ],
            axis=1,
        ).astype(np.float32)
        bv = bf[dm0 : dm0 + 1024, 1].reshape(2, 4, 128)
        bvq = np.zeros((2, 128, 520), dtype=BF16_NP)
        for g in range(2):
            for hq in range(4):
                bvq[g, :, hq * 130 : hq * 130 + 128] = bv[g, hq][None, :].astype(
                    BF16_NP
                )
                bvq[g, :, hq * 130 + 128] = BF16_NP(WS)
        in_maps.append(
            {
                "xt": xt8, "wqk": wqk, "wv": wv,
                "bqk": bqk, "bvq": bvq, "mask": mask,
            }
        )
    return in_maps


def gather_out(results):
    out = np.empty((B, S, D), dtype=np.float32)
    for c in range(N_CORES):
        bcore, hg = c // 2, c % 2
        out[bcore][:, hg * 1024 : hg * 1024 + 1024] = results[c]["out"]
    return out


_CACHED_NC = None


def kernel(hidden_states, W, b):
    global _CACHED_NC
    if _CACHED_NC is None:
        _CACHED_NC = build_nc()
    in_maps = make_in_maps(hidden_states, W, b)
    res = run_bass_kernel_spmd(_CACHED_NC, in_maps, core_ids=list(range(N_CORES)))
    return gather_out(res.results)
